# revision 24
# baseline (speedup 1.0000x reference)
"""Trainium2 Bass kernel for BioSelfAttention (LIF firing rates + winner-take-all).

Math notes (validated against the jax reference on host):
  * LIF with constant input J and exact reset-to-zero is exactly periodic: the
    spike count over N=100 steps is floor(N / k1) with
    k1 = ceil(ln(1-1/J)/ln(0.95)) (0 spikes if J <= 1 or k1 > N).
    ln(1-1/J) = ln(J-1) - ln(J) avoids a division; floor/ceil are computed
    exactly in f32 with the 2^23 round-to-nearest trick plus a compare, and
    floor(100/k1) via an approximate reciprocal candidate fixed up with one
    exact integer comparison (all products < 2^24 are exact in f32).
  * The WTA matrix W = inh*ones + (exc-inh)*I, so x @ W.T = inh*sum(x) + 2*x.
    The per-pair sum is computed on the PE with a constant -0.9 ones matrix
    (bf16), which also broadcasts it across partitions.  Each iteration is ONE
    fused custom-DVE op per pair: x <- clip(3x + nS, 0, 1) with the row-sums
    for the next iteration coming out of the same instruction's accumulator.
  * Early exit: x=0 is absorbing for the WTA update (clip(3*0-0.9*S)=0), and
    stage-1 collapse propagates exactly: rates1=0 => J2=0 => LIF(0)=0 => WTA2
    stays 0.  With inhibition 0.9 and n=128 units, stage 1 collapses to
    exactly zero in iteration 1 whenever a pair's rate sum exceeds ~3.4 --
    the typical case for this module's hyperparameters.

    The kernel exploits this with a two-program scheme (an on-device branch
    would pay ~8us of semaphore-compensation at the CFG merge, so the branch
    lives on the host instead):
      NEFF-A (always runs, branchless): J1 = <Q,K>, stage-1 LIF rates, WTA
        iteration 1.  Writes OUT = all zeros EXCEPT the iteration-1 state
        x1 (128x4 per core), stored into fixed positions of OUT.
        If the state collapsed, x1 == 0 exactly, so OUT is all zeros --
        the mathematically exact final answer.
      Host: if OUT has any nonzero (state survived), run NEFF-B -- the
        full unconditional pipeline (both WTA stages, stage-2 LIF) -- and
        return its OUT instead.
  * Work is data-parallel over the B*H = 32 (batch, head) pairs: 4 per core.

Layout per core: SBUF tiles are (T=128 partitions, S=4 pairs, D=64).
"""

import math

import numpy as np

_B, _H, _T, _D = 4, 8, 128, 64
_NCORES = 8
_S = (_B * _H) // _NCORES  # (b,h) pairs per core = 4

_DECAY = 1.0 - 0.001 / 0.02  # 0.95
_WTA_INH = -0.9
_WTA_STEPS = 20

_MAGIC = 8388608.0  # 2^23: (y + MAGIC) - MAGIC == round-to-nearest-even(y)
_EPS = 1e-30
_CLN = 1.0 / math.log(_DECAY)

_cache = {}


def _f32(x):
    return np.asarray(x, np.float32) if isinstance(x, np.ndarray) else np.float32(x)


def _register_dve_ops():
    """Append the fused ops this kernel uses to the custom-DVE registry."""
    import concourse.dve_ops as D
    from concourse.dve_spec import (
        Spec, Src0, Src1, C0, C1, C2, Zero, One, maxx, minn, lower,
    )
    from concourse.dve_spec import _has_src1 as has_src1
    from concourse.dve_uop import DveOpSpec, AluOp

    if "BIO_WTA_STEP_T" in D._SUB_OPCODE_FOR_NAME:
        return D

    def add_op(name, spec, subdim=False):
        row = D._CUSTOM_DVE_ROW_BASE + len(D.OPS)
        assert row < 0x20
        D._SUB_OPCODE_FOR_NAME[name] = row
        shas = {}
        for ver in ("v3", "v4"):
            try:
                res = DveOpSpec(
                    name=name, opcode=row, uops=lower(spec, ver=ver),
                    rd1_en=has_src1(spec),
                )
                shas[ver] = res.sha(ver)
            except Exception:
                pass
        op = D.DveOp(name, spec, subdim, shas)
        D.OPS.append(op)
        D.CUSTOM_DVE_SPECS[name] = spec
        return op

    F = _f32

    # row-dot: out = in0*in1 elementwise, accum_out = row-sum of products
    add_op("BIO_DOT", Spec(
        body=Src0 * Src1,
        accum=AluOp.ADD,
        reference=lambda in0, in1, s0, s1, imm2: (lambda o: (
            o, o.sum(-1, keepdims=True, dtype=np.float32)))(F(F(in0) * F(in1))),
    ))
    # x <- clip(x*s0 + nS, 0, 1); nS arrives as a same-shape stream (in1)
    add_op("BIO_WTA_STEP_T", Spec(
        body=minn(maxx(Src0 * C0 + Src1, Zero), One),
        reference=lambda in0, in1, s0, s1, imm2: np.clip(
            F(F(F(in0) * F(s0)) + F(in1)), 0.0, 1.0),
    ))
    # x <- clip(x*s0 + nS[p], 0, 1), accum_out = row-sum of the clipped x
    add_op("BIO_WTA_STEP_A", Spec(
        body=minn(maxx(Src0 * C0 + C1, Zero), One),
        accum=AluOp.ADD,
        reference=lambda in0, in1, s0, s1, imm2: (lambda o: (o, o.sum(-1, keepdims=True, dtype=np.float32)))(
            np.clip(F(F(F(in0) * F(s0)) + F(s1)), 0.0, 1.0)),
    ))
    # k1 = ceil(max((lt - lj)*C, 0.5)) in one op: magic-rne then +[y > i0]
    def _yceil_ref(in0, in1, s0, s1, imm2):
        y = np.maximum(F(F(F(in0) - F(in1)) * F(s0)), F(s1))
        i0 = F(F(y + F(imm2)) - F(imm2))
        return F(i0 + F(y > i0))
    def _yceil_body():
        y = maxx((Src0 - Src1) * C0, C1)
        i0 = (y + C2) - C2
        return i0 + (y > i0)
    add_op("BIO_LIF_YCEIL", Spec(body=_yceil_body(), reference=_yceil_ref))
    # y = clamp((lt - lj)*C, 0.5, 1000)
    add_op("BIO_LIF_Y", Spec(
        body=minn(maxx((Src0 - Src1) * C0, C1), C2),
        reference=lambda in0, in1, s0, s1, imm2: np.minimum(
            np.maximum(F(F(F(in0) - F(in1)) * F(s0)), F(s1)), F(imm2)),
    ))
    # k1 = ceil(y) exactly: i0 = rne(y) via magic add/sub, then +[y > i0]
    def _ceil_ref(in0, in1, s0, s1, imm2):
        i0 = F(F(F(in0) + F(s0)) - F(s0))
        return F(i0 + F(F(in0) > i0))
    add_op("BIO_LIF_CEIL", Spec(
        body=(lambda i0: i0 + (Src0 > i0))((Src0 + C0) - C0),
        reference=_ceil_ref,
    ))
    # cc = floor(100/k1) exactly from approximate r ~ 1/k1 (in0) and k1 (in1):
    # c0m1 = rne(100 r) - 1;  cc = c0m1 + [ (c0m1+1)*k1 <= 100 ]
    def _cnt_ref(in0, in1, s0, s1, imm2):
        p = F(F(in0) * F(s0))
        c0m1 = F(F(p + F(s1)) - F(imm2))
        m1 = F(F(c0m1 + np.float32(1.0)) * F(in1))
        return F(c0m1 + F(m1 <= F(s0)))
    def _cnt_body():
        p = Src0 * C0
        c0m1 = (p + C1) - C2
        m1 = (c0m1 + One) * Src1
        return c0m1 + (m1 <= C0)
    add_op("BIO_LIF_CNT", Spec(body=_cnt_body(), reference=_cnt_ref))
    # rate = (cc*s0) * [J > s1]
    add_op("BIO_LIF_RATE", Spec(
        body=(Src0 * C0) * (Src1 > C1),
        reference=lambda in0, in1, s0, s1, imm2: F(
            F(F(in0) * F(s0)) * F(F(in1) > F(s1))),
    ))
    # same, plus accum_out = row-sum of the rates (seeds the WTA accumulator)
    add_op("BIO_LIF_RATE_ACC", Spec(
        body=(Src0 * C0) * (Src1 > C1),
        accum=AluOp.ADD,
        reference=lambda in0, in1, s0, s1, imm2: (lambda o: (
            o, o.sum(-1, keepdims=True, dtype=np.float32)))(
                F(F(F(in0) * F(s0)) * F(F(in1) > F(s1)))),
    ))
    return D


# Asymmetric clamps make the [J > 1] mask unnecessary: for every J <= 1 the
# packed-log path gives y = (ln(max(J-1,EPS_A)) - ln(max(J,EPS_B))) * CLN
# >= (ln(1e-30) - ln(1e-10)) * CLN ~ 898 > 100, so the spike count is exactly
# 0 without masking.  (CLN = 1/ln(0.95) is negative; lt - lj <= -46 for all
# J <= 1, and -46 * CLN ~ 898.)
_EPS_A = 1e-30
_EPS_B = 1e-10


def _emit_lif_cnt(nc, pool, mybir, dve, J, F, tag, accum_outs):
    """LIF firing rates for constant input J: (128, F) f32 -> (128, F) f32.

    Returns the rate tile; accum_outs receives the per-partition row sums of
    the rates (seeds the first WTA-step accumulator).  The asymmetric clamps
    (_EPS_A/_EPS_B) already force count 0 for every J <= 1, so the RATE op's
    mask input is fed an always-true condition.

    Narrow inputs pack (J-1 | J) side by side for a single Ln activation.
    Wide inputs are processed in two halves so the ACT-engine Ln latency of
    one half overlaps the Vector-engine tail of the other (Tile schedules by
    data dependencies)."""
    op = mybir.AluOpType
    act = mybir.ActivationFunctionType
    f32 = mybir.dt.float32

    def t(name):
        return pool.tile([128, F], f32, tag=f"{tag}_{name}", name=f"{tag}_{name}")

    k1, r = (t(n) for n in ("k1", "r"))
    cc = t("cc")
    if len(J.shape) == 3:
        J = J.rearrange("p a b -> p (a b)")
    out = pool.tile([128, F], f32, tag=f"{tag}_out", name=f"{tag}_out")
    if F <= 64:
        # narrow input: pack (J-1 | J) side by side and take ONE Ln over both
        # (the ACT fixed cost dominates at this width)
        tj = pool.tile([128, 2 * F], f32, tag=f"{tag}_tj", name=f"{tag}_tj")
        lb = pool.tile([128, 2 * F], f32, tag=f"{tag}_lb", name=f"{tag}_lb")
        nc.vector.tensor_scalar(tj[:, 0:F], J, 1.0, _EPS_A, op.subtract, op.max)
        nc.vector.tensor_scalar(tj[:, F:2 * F], J, _EPS_B, None, op.max)
        nc.scalar.activation(lb[:], tj[:], act.Ln)
        nc.vector._custom_dve(dve["BIO_LIF_YCEIL"], out=k1[:], in0=lb[:, 0:F],
                              in1=lb[:, F:2 * F], s0=_CLN, s1=0.5, imm2=_MAGIC)
        nc.vector.reciprocal_approx_fast(out=r[:], in_=k1[:])
        nc.vector._custom_dve(dve["BIO_LIF_CNT"], out=cc[:], in0=r[:],
                              in1=k1[:], s0=100.0, s1=_MAGIC, imm2=_MAGIC + 1.0)
        nc.vector._custom_dve(dve["BIO_LIF_RATE_ACC"], out=out[:],
                              in0=cc[:], in1=cc[:], s0=0.01, s1=-1.0,
                              accum_out=accum_outs[0])
        return out
    tm1, jc, lt, lj = (t(n) for n in ("tm1", "jc", "lt", "lj"))
    halves = [slice(0, F // 2), slice(F // 2, F)]
    for h in halves:
        nc.vector.tensor_scalar(tm1[:, h], J[:, h], 1.0, _EPS_A,
                                op.subtract, op.max)
        nc.vector.tensor_scalar(jc[:, h], J[:, h], _EPS_B, None, op.max)
        nc.scalar.activation(lt[:, h], tm1[:, h], act.Ln)
        nc.scalar.activation(lj[:, h], jc[:, h], act.Ln)
        nc.vector._custom_dve(dve["BIO_LIF_YCEIL"], out=k1[:, h], in0=lt[:, h],
                              in1=lj[:, h], s0=_CLN, s1=0.5, imm2=_MAGIC)
        nc.vector.reciprocal_approx_fast(out=r[:, h], in_=k1[:, h])
        nc.vector._custom_dve(dve["BIO_LIF_CNT"], out=cc[:, h], in0=r[:, h],
                              in1=k1[:, h], s0=100.0, s1=_MAGIC,
                              imm2=_MAGIC + 1.0)
        nc.vector._custom_dve(dve["BIO_LIF_RATE_ACC"], out=out[:, h],
                              in0=cc[:, h], in1=cc[:, h], s0=0.01, s1=-1.0,
                              accum_out=accum_outs[h.start > 0])
    return out


_A_, _B_pack = 32, 4  # packed layout: t = 4*a + b


def _packed(ap):
    return ap.rearrange("s (a b) d -> (s a) b d", a=_A_, b=_B_pack)


def _emit_stage1(nc, tc, pool, psum_pool, mybir, dve, tq, tk, mb, x1_out=None):
    """J1 = <Q,K> rows -> LIF rates -> WTA iteration 1.

    Returns (x1, accb2): the post-iteration-1 state (128 x 4, f32) and its
    per-partition row sums.  If x1_out is given, the state is written there
    instead of a fresh tile."""
    op = mybir.AluOpType
    f32 = mybir.dt.float32
    bf16 = mybir.dt.bfloat16
    T, B_ = _T, _B_pack

    prod = pool.tile([T, B_, _D], f32)
    nc.vector.tensor_mul(prod[:], tq[:], tk[:])
    j1 = pool.tile([T, B_], f32)
    nc.vector.tensor_reduce(j1[:], prod[:], mybir.AxisListType.X, op.add)

    acc1b = pool.tile([T, 1], bf16)
    x1 = _emit_lif_cnt(nc, pool, mybir, dve, j1[:], B_, "lif1",
                       accum_outs=[acc1b[:]])
    ns1 = psum_pool.tile([T, 1], f32, tag="w1_ns")
    nc.tensor.matmul(ns1[:], mb[:], acc1b[:])
    accb2 = pool.tile([T, 1], f32)
    out_ap = x1[:] if x1_out is None else x1_out
    nc.vector._custom_dve(dve["BIO_WTA_STEP_A"], out=out_ap, in0=x1[:],
                          s0=3.0, s1=ns1[:], accum_out=accb2[:])
    return (x1 if x1_out is None else None), accb2


def _emit_mb(nc, pool, bf16, value, tag):
    """Block-diagonal constant matrix (bf16): matmul of per-partition row
    sums against it yields value * (pair sum) on every partition."""
    mb = pool.tile([128, 128], bf16, tag=tag, name=tag)
    nc.gpsimd.memset(mb[:], 0.0)
    for s in range(_S):
        nc.gpsimd.memset(mb[32 * s: 32 * (s + 1), 32 * s: 32 * (s + 1)],
                         value)
    return mb


def _new_bass():
    import concourse.bacc as bacc

    nc = bacc.Bacc(
        "TRN2",
        target_bir_lowering=False,
        debug=False,
        enable_asserts=False,
        num_devices=_NCORES,
    )
    # Keep data waits on the matmuls instead of their weight loads: the WTA
    # weight matrix is written once, so the per-iteration LDWEIGHTS can run
    # early (overlapping the Vector step) instead of sitting in the serial
    # accb -> matmul chain.
    nc.move_matmul_waits_to_ldweights = lambda: None
    return nc


def _build_fast_nc():
    """NEFF-A: branchless probe.  OUT = zeros except the stage-1 iteration-1
    state x1, written into OUT[s=0..3, t<4(packed), d] positions.  Collapsed
    state => x1 == 0 => OUT is exactly all zeros (the final answer)."""
    import concourse.mybir as mybir
    import concourse.tile as tile

    D_ops = _register_dve_ops()
    dve = {o.name: o for o in D_ops.OPS}
    act = mybir.ActivationFunctionType
    f32 = mybir.dt.float32
    bf16 = mybir.dt.bfloat16
    T, B_ = _T, _B_pack

    nc = _new_bass()
    qd = nc.dram_tensor("Q", (_S, T, _D), f32, kind="ExternalInput").ap()
    kd = nc.dram_tensor("K", (_S, T, _D), f32, kind="ExternalInput").ap()
    vd = nc.dram_tensor("V", (_S, T, _D), f32, kind="ExternalInput").ap()  # noqa: F841 (same I/O signature)
    od = nc.dram_tensor("OUT", (_S, T, _D), f32, kind="ExternalOutput").ap()

    with tile.TileContext(nc) as tc:
        with (
            tc.tile_pool(name="main", bufs=1) as pool,
            tc.tile_pool(name="psum", bufs=2, space="PSUM") as psum_pool,
        ):
            # dummy Ln up front so the ACT table load overlaps the DMAs
            warm = pool.tile([128, 1], f32)
            nc.vector.memset(warm, 1.0)
            nc.scalar.activation(warm, warm, act.Ln)

            tq = pool.tile([T, B_, _D], f32)
            tk = pool.tile([T, B_, _D], f32)
            nc.sync.dma_start(tq[:], _packed(qd))
            nc.scalar.dma_start(tk[:], _packed(kd))

            # Zero the output EARLY in two region DMAs that are disjoint from
            # the state corner, so they run concurrently with the compute and
            # need no ordering against the post-compute corner store.
            zt = pool.tile([T, B_, _D], f32)
            nc.gpsimd.memset(zt[:], 0.0)
            po = _packed(od)
            nc.sync.dma_start(po[:, 0, 4:_D], zt[:, 0, 4:_D])
            nc.scalar.dma_start(po[:, 1:B_, :], zt[:, 1:B_, :])

            mb = _emit_mb(nc, pool, bf16, _WTA_INH, "mb09")

            # stage-1 iteration-1 state -> the 128x4 corner of OUT.
            # Collapsed => x1c == 0 => OUT is exactly all zeros.
            x1c = pool.tile([T, 4], f32)
            _emit_stage1(nc, tc, pool, psum_pool, mybir, dve, tq, tk, mb,
                         x1_out=x1c[:])
            nc.sync.dma_start(po[:, 0, 0:4], x1c[:])

    nc.compile()
    return nc


def _build_slow_nc():
    """NEFF-B: the full unconditional pipeline (both WTA stages)."""
    import concourse.mybir as mybir
    import concourse.tile as tile

    D_ops = _register_dve_ops()
    dve = {o.name: o for o in D_ops.OPS}
    op = mybir.AluOpType
    act = mybir.ActivationFunctionType
    f32 = mybir.dt.float32
    bf16 = mybir.dt.bfloat16
    T, B_, D = _T, _B_pack, _D

    nc = _new_bass()
    qd = nc.dram_tensor("Q", (_S, T, D), f32, kind="ExternalInput").ap()
    kd = nc.dram_tensor("K", (_S, T, D), f32, kind="ExternalInput").ap()
    vd = nc.dram_tensor("V", (_S, T, D), f32, kind="ExternalInput").ap()
    od = nc.dram_tensor("OUT", (_S, T, D), f32, kind="ExternalOutput").ap()

    with tile.TileContext(nc) as tc:
        with (
            tc.tile_pool(name="main", bufs=1) as pool,
            tc.tile_pool(name="psum", bufs=2, space="PSUM") as psum_pool,
        ):
            warm = pool.tile([128, 1], f32)
            nc.vector.memset(warm, 1.0)
            nc.scalar.activation(warm, warm, act.Ln)

            tq = pool.tile([T, B_, D], f32)
            tk = pool.tile([T, B_, D], f32)
            tv = pool.tile([T, B_, D], f32)
            nc.sync.dma_start(tq[:], _packed(qd))
            nc.scalar.dma_start(tk[:], _packed(kd))
            nc.sync.dma_start(tv[:], _packed(vd))

            mb = _emit_mb(nc, pool, bf16, _WTA_INH, "mb09")

            x1, accb2 = _emit_stage1(nc, tc, pool, psum_pool, mybir, dve,
                                     tq, tk, mb)

            def wta_loop(x, accb, tag, steps):
                for _ in range(steps):
                    ns = psum_pool.tile([T, 1], f32, tag=f"{tag}_ns")
                    nc.tensor.matmul(ns[:], mb[:], accb)
                    nc.vector._custom_dve(dve["BIO_WTA_STEP_A"], out=x, in0=x,
                                          s0=3.0, s1=ns[:], accum_out=accb)

            # stage-1 WTA iterations 2..20 (bf16 accumulator from here on)
            acc1c = pool.tile([T, 1], bf16)
            nc.vector.tensor_copy(acc1c[:], accb2[:])
            wta_loop(x1[:], acc1c[:], "w1", _WTA_STEPS - 1)

            # J2[p, b, d] = rates1[p, b] * V[p, b, d]
            jv = pool.tile([T, B_, D], f32)
            x1b3 = x1.rearrange("p (b u) -> p b u", u=1).broadcast_to((T, B_, D))
            nc.vector.tensor_tensor(jv[:], tv[:], x1b3, op.mult)

            # stage-2 LIF rates -> 20 WTA iterations on (128, 256)
            aH0 = pool.tile([T, 1], f32)
            aH1 = pool.tile([T, 1], f32)
            rate2 = _emit_lif_cnt(nc, pool, mybir, dve, jv[:], B_ * D, "lif2",
                                  accum_outs=[aH0[:], aH1[:]])
            x2 = rate2.rearrange("t (b d) -> t b d", d=D)
            acc2b = pool.tile([T, 1], bf16)
            nc.vector.tensor_tensor(acc2b[:], aH0[:], aH1[:], op.add)
            wta_loop(x2, acc2b[:], "w2", _WTA_STEPS)

            nc.sync.dma_start(_packed(od), x2)

    nc.compile()
    return nc


def _get_nc(which):
    if which not in _cache:
        _cache[which] = {"fast": _build_fast_nc, "slow": _build_slow_nc}[which]()
    return _cache[which]


def run(Q, K, V, **spmd_kwargs):
    """Runs the fast probe; falls back to the full pipeline only when the
    stage-1 state survived (OUT has a nonzero).  Returns the BassKernelResults
    whose OUT is the final answer."""
    from concourse.bass_utils import run_bass_kernel_spmd

    Qr = np.ascontiguousarray(Q, dtype=np.float32).reshape(_NCORES, _S, _T, _D)
    Kr = np.ascontiguousarray(K, dtype=np.float32).reshape(_NCORES, _S, _T, _D)
    Vr = np.ascontiguousarray(V, dtype=np.float32).reshape(_NCORES, _S, _T, _D)
    in_maps = [{"Q": Qr[c], "K": Kr[c], "V": Vr[c]} for c in range(_NCORES)]
    cores = list(range(_NCORES))

    res = run_bass_kernel_spmd(_get_nc("fast"), in_maps, core_ids=cores,
                               **spmd_kwargs)
    if any(res.results[c]["OUT"].any() for c in range(_NCORES)):
        res = run_bass_kernel_spmd(_get_nc("slow"), in_maps, core_ids=cores,
                                   **spmd_kwargs)
    return res


def kernel(Q, K, V):
    res = run(Q, K, V)
    out = np.stack([res.results[c]["OUT"] for c in range(_NCORES)])
    return out.reshape(_B, _H, _T, _D)


# revision 25
# speedup vs baseline: 1.0142x; 1.0142x over previous
"""Trainium2 Bass kernel for BioSelfAttention (LIF firing rates + winner-take-all).

Math notes (validated against the jax reference on host):
  * LIF with constant input J and exact reset-to-zero is exactly periodic: the
    spike count over N=100 steps is floor(N / k1) with
    k1 = ceil(ln(1-1/J)/ln(0.95)) (0 spikes if J <= 1 or k1 > N).
    ln(1-1/J) = ln(J-1) - ln(J) avoids a division; floor/ceil are computed
    exactly in f32 with the 2^23 round-to-nearest trick plus a compare, and
    floor(100/k1) via an approximate reciprocal candidate fixed up with one
    exact integer comparison (all products < 2^24 are exact in f32).
  * The WTA matrix W = inh*ones + (exc-inh)*I, so x @ W.T = inh*sum(x) + 2*x.
    The per-pair sum is computed on the PE with a constant -0.9 ones matrix
    (bf16), which also broadcasts it across partitions.  Each iteration is ONE
    fused custom-DVE op per pair: x <- clip(3x + nS, 0, 1) with the row-sums
    for the next iteration coming out of the same instruction's accumulator.
  * Early exit: x=0 is absorbing for the WTA update (clip(3*0-0.9*S)=0), and
    stage-1 collapse propagates exactly: rates1=0 => J2=0 => LIF(0)=0 => WTA2
    stays 0.  With inhibition 0.9 and n=128 units, stage 1 collapses to
    exactly zero in iteration 1 whenever a pair's rate sum exceeds ~3.4 --
    the typical case for this module's hyperparameters.

    The kernel exploits this with a two-program scheme (an on-device branch
    would pay ~8us of semaphore-compensation at the CFG merge, so the branch
    lives on the host instead):
      NEFF-A (always runs, branchless): J1 = <Q,K>, stage-1 LIF rates, WTA
        iteration 1.  Writes OUT = all zeros EXCEPT the iteration-1 state
        x1 (128x4 per core), stored into fixed positions of OUT.
        If the state collapsed, x1 == 0 exactly, so OUT is all zeros --
        the mathematically exact final answer.
      Host: if OUT has any nonzero (state survived), run NEFF-B -- the
        full unconditional pipeline (both WTA stages, stage-2 LIF) -- and
        return its OUT instead.
  * Work is data-parallel over the B*H = 32 (batch, head) pairs: 4 per core.

Layout per core: SBUF tiles are (T=128 partitions, S=4 pairs, D=64).
"""

import math

import numpy as np

_B, _H, _T, _D = 4, 8, 128, 64
_NCORES = 8
_S = (_B * _H) // _NCORES  # (b,h) pairs per core = 4

_DECAY = 1.0 - 0.001 / 0.02  # 0.95
_WTA_INH = -0.9
_WTA_STEPS = 20

_MAGIC = 8388608.0  # 2^23: (y + MAGIC) - MAGIC == round-to-nearest-even(y)
_EPS = 1e-30
_CLN = 1.0 / math.log(_DECAY)

_cache = {}


def _f32(x):
    return np.asarray(x, np.float32) if isinstance(x, np.ndarray) else np.float32(x)


def _register_dve_ops():
    """Append the fused ops this kernel uses to the custom-DVE registry."""
    import concourse.dve_ops as D
    from concourse.dve_spec import (
        Spec, Src0, Src1, C0, C1, C2, Zero, One, maxx, minn, lower,
    )
    from concourse.dve_spec import _has_src1 as has_src1
    from concourse.dve_uop import DveOpSpec, AluOp

    if "BIO_WTA_STEP_T" in D._SUB_OPCODE_FOR_NAME:
        return D

    def add_op(name, spec, subdim=False):
        row = D._CUSTOM_DVE_ROW_BASE + len(D.OPS)
        assert row < 0x20
        D._SUB_OPCODE_FOR_NAME[name] = row
        shas = {}
        for ver in ("v3", "v4"):
            try:
                res = DveOpSpec(
                    name=name, opcode=row, uops=lower(spec, ver=ver),
                    rd1_en=has_src1(spec),
                )
                shas[ver] = res.sha(ver)
            except Exception:
                pass
        op = D.DveOp(name, spec, subdim, shas)
        D.OPS.append(op)
        D.CUSTOM_DVE_SPECS[name] = spec
        return op

    F = _f32

    # row-dot: out = in0*in1 elementwise, accum_out = row-sum of products
    add_op("BIO_DOT", Spec(
        body=Src0 * Src1,
        accum=AluOp.ADD,
        reference=lambda in0, in1, s0, s1, imm2: (lambda o: (
            o, o.sum(-1, keepdims=True, dtype=np.float32)))(F(F(in0) * F(in1))),
    ))
    # x <- clip(x*s0 + nS, 0, 1); nS arrives as a same-shape stream (in1)
    add_op("BIO_WTA_STEP_T", Spec(
        body=minn(maxx(Src0 * C0 + Src1, Zero), One),
        reference=lambda in0, in1, s0, s1, imm2: np.clip(
            F(F(F(in0) * F(s0)) + F(in1)), 0.0, 1.0),
    ))
    # x <- clip(x*s0 + nS[p], 0, 1), accum_out = row-sum of the clipped x
    add_op("BIO_WTA_STEP_A", Spec(
        body=minn(maxx(Src0 * C0 + C1, Zero), One),
        accum=AluOp.ADD,
        reference=lambda in0, in1, s0, s1, imm2: (lambda o: (o, o.sum(-1, keepdims=True, dtype=np.float32)))(
            np.clip(F(F(F(in0) * F(s0)) + F(s1)), 0.0, 1.0)),
    ))
    # k1 = ceil(max((lt - lj)*C, 0.5)) in one op: magic-rne then +[y > i0]
    def _yceil_ref(in0, in1, s0, s1, imm2):
        y = np.maximum(F(F(F(in0) - F(in1)) * F(s0)), F(s1))
        i0 = F(F(y + F(imm2)) - F(imm2))
        return F(i0 + F(y > i0))
    def _yceil_body():
        y = maxx((Src0 - Src1) * C0, C1)
        i0 = (y + C2) - C2
        return i0 + (y > i0)
    add_op("BIO_LIF_YCEIL", Spec(body=_yceil_body(), reference=_yceil_ref))
    # y = clamp((lt - lj)*C, 0.5, 1000)
    add_op("BIO_LIF_Y", Spec(
        body=minn(maxx((Src0 - Src1) * C0, C1), C2),
        reference=lambda in0, in1, s0, s1, imm2: np.minimum(
            np.maximum(F(F(F(in0) - F(in1)) * F(s0)), F(s1)), F(imm2)),
    ))
    # k1 = ceil(y) exactly: i0 = rne(y) via magic add/sub, then +[y > i0]
    def _ceil_ref(in0, in1, s0, s1, imm2):
        i0 = F(F(F(in0) + F(s0)) - F(s0))
        return F(i0 + F(F(in0) > i0))
    add_op("BIO_LIF_CEIL", Spec(
        body=(lambda i0: i0 + (Src0 > i0))((Src0 + C0) - C0),
        reference=_ceil_ref,
    ))
    # cc = floor(100/k1) exactly from approximate r ~ 1/k1 (in0) and k1 (in1):
    # c0m1 = rne(100 r) - 1;  cc = c0m1 + [ (c0m1+1)*k1 <= 100 ]
    def _cnt_ref(in0, in1, s0, s1, imm2):
        p = F(F(in0) * F(s0))
        c0m1 = F(F(p + F(s1)) - F(imm2))
        m1 = F(F(c0m1 + np.float32(1.0)) * F(in1))
        return F(c0m1 + F(m1 <= F(s0)))
    def _cnt_body():
        p = Src0 * C0
        c0m1 = (p + C1) - C2
        m1 = (c0m1 + One) * Src1
        return c0m1 + (m1 <= C0)
    add_op("BIO_LIF_CNT", Spec(body=_cnt_body(), reference=_cnt_ref))
    # rate = (cc*s0) * [J > s1]
    add_op("BIO_LIF_RATE", Spec(
        body=(Src0 * C0) * (Src1 > C1),
        reference=lambda in0, in1, s0, s1, imm2: F(
            F(F(in0) * F(s0)) * F(F(in1) > F(s1))),
    ))
    # same, plus accum_out = row-sum of the rates (seeds the WTA accumulator)
    add_op("BIO_LIF_RATE_ACC", Spec(
        body=(Src0 * C0) * (Src1 > C1),
        accum=AluOp.ADD,
        reference=lambda in0, in1, s0, s1, imm2: (lambda o: (
            o, o.sum(-1, keepdims=True, dtype=np.float32)))(
                F(F(F(in0) * F(s0)) * F(F(in1) > F(s1)))),
    ))
    return D


# Asymmetric clamps make the [J > 1] mask unnecessary: for every J <= 1 the
# packed-log path gives y = (ln(max(J-1,EPS_A)) - ln(max(J,EPS_B))) * CLN
# >= (ln(1e-30) - ln(1e-10)) * CLN ~ 898 > 100, so the spike count is exactly
# 0 without masking.  (CLN = 1/ln(0.95) is negative; lt - lj <= -46 for all
# J <= 1, and -46 * CLN ~ 898.)
_EPS_A = 1e-30
_EPS_B = 1e-10


def _emit_lif_cnt(nc, pool, mybir, dve, J, F, tag, accum_outs):
    """LIF firing rates for constant input J: (128, F) f32 -> (128, F) f32.

    Returns the rate tile; accum_outs receives the per-partition row sums of
    the rates (seeds the first WTA-step accumulator).  The asymmetric clamps
    (_EPS_A/_EPS_B) already force count 0 for every J <= 1, so the RATE op's
    mask input is fed an always-true condition.

    Narrow inputs pack (J-1 | J) side by side for a single Ln activation.
    Wide inputs are processed in two halves so the ACT-engine Ln latency of
    one half overlaps the Vector-engine tail of the other (Tile schedules by
    data dependencies)."""
    op = mybir.AluOpType
    act = mybir.ActivationFunctionType
    f32 = mybir.dt.float32

    def t(name):
        return pool.tile([128, F], f32, tag=f"{tag}_{name}", name=f"{tag}_{name}")

    k1, r = (t(n) for n in ("k1", "r"))
    cc = t("cc")
    if len(J.shape) == 3:
        J = J.rearrange("p a b -> p (a b)")
    out = pool.tile([128, F], f32, tag=f"{tag}_out", name=f"{tag}_out")
    if F <= 64:
        # narrow input: pack (J-1 | J) side by side and take ONE Ln over both
        # (the ACT fixed cost dominates at this width)
        tj = pool.tile([128, 2 * F], f32, tag=f"{tag}_tj", name=f"{tag}_tj")
        lb = pool.tile([128, 2 * F], f32, tag=f"{tag}_lb", name=f"{tag}_lb")
        nc.vector.tensor_scalar(tj[:, 0:F], J, 1.0, _EPS_A, op.subtract, op.max)
        nc.vector.tensor_scalar(tj[:, F:2 * F], J, _EPS_B, None, op.max)
        nc.scalar.activation(lb[:], tj[:], act.Ln)
        nc.vector._custom_dve(dve["BIO_LIF_YCEIL"], out=k1[:], in0=lb[:, 0:F],
                              in1=lb[:, F:2 * F], s0=_CLN, s1=0.5, imm2=_MAGIC)
        nc.vector.reciprocal_approx_fast(out=r[:], in_=k1[:])
        nc.vector._custom_dve(dve["BIO_LIF_CNT"], out=cc[:], in0=r[:],
                              in1=k1[:], s0=100.0, s1=_MAGIC, imm2=_MAGIC + 1.0)
        nc.vector._custom_dve(dve["BIO_LIF_RATE_ACC"], out=out[:],
                              in0=cc[:], in1=cc[:], s0=0.01, s1=-1.0,
                              accum_out=accum_outs[0])
        return out
    tm1, jc, lt, lj = (t(n) for n in ("tm1", "jc", "lt", "lj"))
    halves = [slice(0, F // 2), slice(F // 2, F)]
    for h in halves:
        nc.vector.tensor_scalar(tm1[:, h], J[:, h], 1.0, _EPS_A,
                                op.subtract, op.max)
        nc.vector.tensor_scalar(jc[:, h], J[:, h], _EPS_B, None, op.max)
        nc.scalar.activation(lt[:, h], tm1[:, h], act.Ln)
        nc.scalar.activation(lj[:, h], jc[:, h], act.Ln)
        nc.vector._custom_dve(dve["BIO_LIF_YCEIL"], out=k1[:, h], in0=lt[:, h],
                              in1=lj[:, h], s0=_CLN, s1=0.5, imm2=_MAGIC)
        nc.vector.reciprocal_approx_fast(out=r[:, h], in_=k1[:, h])
        nc.vector._custom_dve(dve["BIO_LIF_CNT"], out=cc[:, h], in0=r[:, h],
                              in1=k1[:, h], s0=100.0, s1=_MAGIC,
                              imm2=_MAGIC + 1.0)
        nc.vector._custom_dve(dve["BIO_LIF_RATE_ACC"], out=out[:, h],
                              in0=cc[:, h], in1=cc[:, h], s0=0.01, s1=-1.0,
                              accum_out=accum_outs[h.start > 0])
    return out


_A_, _B_pack = 32, 4  # packed layout: t = 4*a + b


def _packed(ap):
    return ap.rearrange("s (a b) d -> (s a) b d", a=_A_, b=_B_pack)


def _emit_stage1(nc, tc, pool, psum_pool, mybir, dve, tq, tk, mb, x1_out=None):
    """J1 = <Q,K> rows -> LIF rates -> WTA iteration 1.

    Returns (x1, accb2): the post-iteration-1 state (128 x 4, f32) and its
    per-partition row sums.  If x1_out is given, the state is written there
    instead of a fresh tile."""
    op = mybir.AluOpType
    f32 = mybir.dt.float32
    bf16 = mybir.dt.bfloat16
    T, B_ = _T, _B_pack

    prod = pool.tile([T, B_, _D], f32)
    nc.vector.tensor_mul(prod[:], tq[:], tk[:])
    j1 = pool.tile([T, B_], f32)
    nc.vector.tensor_reduce(j1[:], prod[:], mybir.AxisListType.X, op.add)

    acc1b = pool.tile([T, 1], bf16)
    x1 = _emit_lif_cnt(nc, pool, mybir, dve, j1[:], B_, "lif1",
                       accum_outs=[acc1b[:]])
    ns1 = psum_pool.tile([T, 1], f32, tag="w1_ns")
    nc.tensor.matmul(ns1[:], mb[:], acc1b[:])
    if x1_out is not None:
        # fast probe: the row-sum accumulator is not needed (the host reads
        # the state itself), so use the accumulator-free step op with the
        # per-partition nS broadcast across the free dim
        ns1b = ns1.rearrange("p (a u) -> p a u", u=1).broadcast_to((T, 1, B_))
        nc.vector._custom_dve(dve["BIO_WTA_STEP_T"], out=x1_out, in0=x1[:],
                              s0=3.0, in1=ns1b)
        return None, None
    accb2 = pool.tile([T, 1], f32)
    nc.vector._custom_dve(dve["BIO_WTA_STEP_A"], out=x1[:], in0=x1[:],
                          s0=3.0, s1=ns1[:], accum_out=accb2[:])
    return x1, accb2


def _emit_mb(nc, pool, bf16, value, tag):
    """Block-diagonal constant matrix (bf16): matmul of per-partition row
    sums against it yields value * (pair sum) on every partition."""
    mb = pool.tile([128, 128], bf16, tag=tag, name=tag)
    nc.gpsimd.memset(mb[:], 0.0)
    for s in range(_S):
        nc.gpsimd.memset(mb[32 * s: 32 * (s + 1), 32 * s: 32 * (s + 1)],
                         value)
    return mb


def _new_bass():
    import concourse.bacc as bacc

    nc = bacc.Bacc(
        "TRN2",
        target_bir_lowering=False,
        debug=False,
        enable_asserts=False,
        num_devices=_NCORES,
    )
    # Keep data waits on the matmuls instead of their weight loads: the WTA
    # weight matrix is written once, so the per-iteration LDWEIGHTS can run
    # early (overlapping the Vector step) instead of sitting in the serial
    # accb -> matmul chain.
    nc.move_matmul_waits_to_ldweights = lambda: None
    return nc


def _build_fast_nc():
    """NEFF-A: branchless probe.  OUT = zeros except the stage-1 iteration-1
    state x1, written into OUT[s=0..3, t<4(packed), d] positions.  Collapsed
    state => x1 == 0 => OUT is exactly all zeros (the final answer)."""
    import concourse.mybir as mybir
    import concourse.tile as tile

    D_ops = _register_dve_ops()
    dve = {o.name: o for o in D_ops.OPS}
    act = mybir.ActivationFunctionType
    f32 = mybir.dt.float32
    bf16 = mybir.dt.bfloat16
    T, B_ = _T, _B_pack

    nc = _new_bass()
    qd = nc.dram_tensor("Q", (_S, T, _D), f32, kind="ExternalInput").ap()
    kd = nc.dram_tensor("K", (_S, T, _D), f32, kind="ExternalInput").ap()
    vd = nc.dram_tensor("V", (_S, T, _D), f32, kind="ExternalInput").ap()  # noqa: F841 (same I/O signature)
    od = nc.dram_tensor("OUT", (_S, T, _D), f32, kind="ExternalOutput").ap()

    with tile.TileContext(nc) as tc:
        with (
            tc.tile_pool(name="main", bufs=1) as pool,
            tc.tile_pool(name="psum", bufs=2, space="PSUM") as psum_pool,
        ):
            # dummy Ln up front so the ACT table load overlaps the DMAs
            warm = pool.tile([128, 1], f32)
            nc.vector.memset(warm, 1.0)
            nc.scalar.activation(warm, warm, act.Ln)

            tq = pool.tile([T, B_, _D], f32)
            tk = pool.tile([T, B_, _D], f32)
            nc.sync.dma_start(tq[:], _packed(qd))
            nc.scalar.dma_start(tk[:], _packed(kd))

            # Zero the output EARLY in two region DMAs that are disjoint from
            # the state corner, so they run concurrently with the compute and
            # need no ordering against the post-compute corner store.
            zt = pool.tile([T, B_, _D], f32)
            nc.gpsimd.memset(zt[:], 0.0)
            po = _packed(od)
            nc.sync.dma_start(po[:, 0, 4:_D], zt[:, 0, 4:_D])
            nc.scalar.dma_start(po[:, 1:B_, :], zt[:, 1:B_, :])

            mb = _emit_mb(nc, pool, bf16, _WTA_INH, "mb09")

            # stage-1 iteration-1 state -> the 128x4 corner of OUT.
            # Collapsed => x1c == 0 => OUT is exactly all zeros.
            x1c = pool.tile([T, 4], f32)
            _emit_stage1(nc, tc, pool, psum_pool, mybir, dve, tq, tk, mb,
                         x1_out=x1c[:])
            nc.sync.dma_start(po[:, 0, 0:4], x1c[:])

    nc.compile()
    return nc


def _build_slow_nc():
    """NEFF-B: the full unconditional pipeline (both WTA stages)."""
    import concourse.mybir as mybir
    import concourse.tile as tile

    D_ops = _register_dve_ops()
    dve = {o.name: o for o in D_ops.OPS}
    op = mybir.AluOpType
    act = mybir.ActivationFunctionType
    f32 = mybir.dt.float32
    bf16 = mybir.dt.bfloat16
    T, B_, D = _T, _B_pack, _D

    nc = _new_bass()
    qd = nc.dram_tensor("Q", (_S, T, D), f32, kind="ExternalInput").ap()
    kd = nc.dram_tensor("K", (_S, T, D), f32, kind="ExternalInput").ap()
    vd = nc.dram_tensor("V", (_S, T, D), f32, kind="ExternalInput").ap()
    od = nc.dram_tensor("OUT", (_S, T, D), f32, kind="ExternalOutput").ap()

    with tile.TileContext(nc) as tc:
        with (
            tc.tile_pool(name="main", bufs=1) as pool,
            tc.tile_pool(name="psum", bufs=2, space="PSUM") as psum_pool,
        ):
            warm = pool.tile([128, 1], f32)
            nc.vector.memset(warm, 1.0)
            nc.scalar.activation(warm, warm, act.Ln)

            tq = pool.tile([T, B_, D], f32)
            tk = pool.tile([T, B_, D], f32)
            tv = pool.tile([T, B_, D], f32)
            nc.sync.dma_start(tq[:], _packed(qd))
            nc.scalar.dma_start(tk[:], _packed(kd))
            nc.sync.dma_start(tv[:], _packed(vd))

            mb = _emit_mb(nc, pool, bf16, _WTA_INH, "mb09")

            x1, accb2 = _emit_stage1(nc, tc, pool, psum_pool, mybir, dve,
                                     tq, tk, mb)

            def wta_loop(x, accb, tag, steps):
                for _ in range(steps):
                    ns = psum_pool.tile([T, 1], f32, tag=f"{tag}_ns")
                    nc.tensor.matmul(ns[:], mb[:], accb)
                    nc.vector._custom_dve(dve["BIO_WTA_STEP_A"], out=x, in0=x,
                                          s0=3.0, s1=ns[:], accum_out=accb)

            # stage-1 WTA iterations 2..20 (bf16 accumulator from here on)
            acc1c = pool.tile([T, 1], bf16)
            nc.vector.tensor_copy(acc1c[:], accb2[:])
            wta_loop(x1[:], acc1c[:], "w1", _WTA_STEPS - 1)

            # J2[p, b, d] = rates1[p, b] * V[p, b, d]
            jv = pool.tile([T, B_, D], f32)
            x1b3 = x1.rearrange("p (b u) -> p b u", u=1).broadcast_to((T, B_, D))
            nc.vector.tensor_tensor(jv[:], tv[:], x1b3, op.mult)

            # stage-2 LIF rates -> 20 WTA iterations on (128, 256)
            aH0 = pool.tile([T, 1], f32)
            aH1 = pool.tile([T, 1], f32)
            rate2 = _emit_lif_cnt(nc, pool, mybir, dve, jv[:], B_ * D, "lif2",
                                  accum_outs=[aH0[:], aH1[:]])
            x2 = rate2.rearrange("t (b d) -> t b d", d=D)
            acc2b = pool.tile([T, 1], bf16)
            nc.vector.tensor_tensor(acc2b[:], aH0[:], aH1[:], op.add)
            wta_loop(x2, acc2b[:], "w2", _WTA_STEPS)

            nc.sync.dma_start(_packed(od), x2)

    nc.compile()
    return nc


def _get_nc(which):
    if which not in _cache:
        _cache[which] = {"fast": _build_fast_nc, "slow": _build_slow_nc}[which]()
    return _cache[which]


def run(Q, K, V, **spmd_kwargs):
    """Runs the fast probe; falls back to the full pipeline only when the
    stage-1 state survived (OUT has a nonzero).  Returns the BassKernelResults
    whose OUT is the final answer."""
    from concourse.bass_utils import run_bass_kernel_spmd

    Qr = np.ascontiguousarray(Q, dtype=np.float32).reshape(_NCORES, _S, _T, _D)
    Kr = np.ascontiguousarray(K, dtype=np.float32).reshape(_NCORES, _S, _T, _D)
    Vr = np.ascontiguousarray(V, dtype=np.float32).reshape(_NCORES, _S, _T, _D)
    in_maps = [{"Q": Qr[c], "K": Kr[c], "V": Vr[c]} for c in range(_NCORES)]
    cores = list(range(_NCORES))

    res = run_bass_kernel_spmd(_get_nc("fast"), in_maps, core_ids=cores,
                               **spmd_kwargs)
    if any(res.results[c]["OUT"].any() for c in range(_NCORES)):
        res = run_bass_kernel_spmd(_get_nc("slow"), in_maps, core_ids=cores,
                                   **spmd_kwargs)
    return res


def kernel(Q, K, V):
    res = run(Q, K, V)
    out = np.stack([res.results[c]["OUT"] for c in range(_NCORES)])
    return out.reshape(_B, _H, _T, _D)
